# revision 74
# baseline (speedup 1.0000x reference)
"""CompressionAttention Trainium2 kernel (8 NeuronCores, SPMD).

Sharding: core i handles batch b=i//4 and 4 heads hh=i%4 (model-dim slice
hh*256:(hh+1)*256). Heads never interact before out_proj, so each core
computes a partial out-projection for its batch; the host gather sums the
4 partials per batch and adds bo once.

Algorithm per core (chunked linear attention, chunk T=128):
  w[c,t] = exp(qc_c . k_t)            (max-subtraction dropped: att is
                                       invariant to per-c scaling of w)
  den[c,s]   = cumsum_t<=s w[c,t]
  att[c,s]   = (w_chunk^T (U o G) + SK^T qs) / den   per chunk + state
  p = softmax_c att ; o_s = sum_c p * (sum_t<=s w v)/den

Schedule: all cross-chunk recurrences (den carry, SK/SV prefixes) and all
G = k^T q blocks are precomputed in side phases, so the main chunk loop has
no chunk->chunk dependency and the Tensor engine never idles long enough to
HAM-throttle. Softmax elementwise ops run in a partition-packed (32h+c, t)
layout (engine time ~ free-size, so 4x cheaper than (c, 4h*t)); the packed
e/pdd are re-laid-out for their matmul consumers with tiny SBUF-to-SBUF
DMAs. 1/x runs as exp(-ln(x)) on Act (single act table, den carry folded
into the ln bias); 1/sums o-normalization is deferred out of the loop.
"""
import os
import numpy as np
import ml_dtypes

import concourse.bacc as bacc
import concourse.tile as tile
from concourse import mybir
from concourse.bass_utils import run_bass_kernel_spmd

F32 = mybir.dt.float32
BF16 = mybir.dt.bfloat16
EXP = mybir.ActivationFunctionType.Exp
LN = mybir.ActivationFunctionType.Ln
IDENT = mybir.ActivationFunctionType.Identity

S, B, D = 2048, 2, 1024
H, DH, C = 16, 64, 16
HPC = 4            # heads per core
HD = HPC * DH      # 256 model dims per core
T = 128            # chunk
NCH = S // T       # 16 chunks
KT = D // 128      # 8 k-tiles of the model dim

LAST_EXEC_NS = None
_CACHE = {}


def _patched_insert_act_table_loads(self):
    """Force every activation (Exp/Ln/Copy/Identity) onto the single
    act-func table that contains them all. The default greedy pass picks
    exp_and_others for Exp and natural_log_exp_and_others for Ln, inserting
    an ACT_TABLE_LOAD (1.28us!) on every switch."""
    import bass_rust as _bass_rust
    from concourse.hw_specs import get_activation_tables
    has_activation = any(
        isinstance(i, mybir.InstActivation)
        for b in self.main_func.blocks
        for i in b.instructions)
    if not has_activation:
        return
    keep = {mybir.ActivationFunctionType.Exp, mybir.ActivationFunctionType.Ln,
            mybir.ActivationFunctionType.Copy,
            mybir.ActivationFunctionType.Identity}
    tables = []
    for name, funcs in get_activation_tables(self.m.arch).items():
        if name != "natural_log_exp_and_others":
            funcs = funcs - keep
        tables.append((name, funcs))
    _bass_rust.insert_act_table_loads(self, tables)


def _build():
    nc = bacc.Bacc("TRN2", target_bir_lowering=False, debug=False, num_devices=8)
    import types as _types
    nc.insert_act_table_loads = _types.MethodType(
        _patched_insert_act_table_loads, nc)

    # ---- DRAM I/O (per-core, host-prepped layouts; bf16 where possible) ----
    xT_d = nc.dram_tensor("xT", [D, S], BF16, kind="ExternalInput")
    wqT_d = nc.dram_tensor("wqT", [D, HD], BF16, kind="ExternalInput")
    wkT_d = nc.dram_tensor("wkT", [D, HD], BF16, kind="ExternalInput")
    wvT_d = nc.dram_tensor("wvT", [D, HD], BF16, kind="ExternalInput")
    woT_d = nc.dram_tensor("woT", [HD, D], BF16, kind="ExternalInput")
    # qc transposed per head, zero-padded 16 -> 32 so the packed wT tile has
    # no uninitialized columns (pad logits are 0 -> w=1, never read).
    qcT_d = nc.dram_tensor("qcT", [64, HPC, 32], BF16, kind="ExternalInput")
    bq_d = nc.dram_tensor("bq", [HD], F32, kind="ExternalInput")
    bk_d = nc.dram_tensor("bk", [HD], F32, kind="ExternalInput")
    bv_d = nc.dram_tensor("bv", [1, HD], BF16, kind="ExternalInput")
    out_d = nc.dram_tensor("out_p", [S, D], BF16, kind="ExternalOutput")

    # ---- consts baked into the NEFF ----
    u = np.triu(np.ones((T, T), np.float32))
    u4 = np.tile(u, (1, 4)).astype(ml_dtypes.bfloat16)
    u4_d = nc.inline_tensor(u4, "u4c")
    # sel2[pair][p, j] = 1 iff p selects head 2*pair (j<64) / 2*pair+1
    # (j>=64): the sums-broadcast matmul consumes packed (32h+c) e directly
    # and emits both heads of a column-block in one matmul.
    sel2 = np.zeros((2, 128, 128), np.float32)
    for pair in range(2):
        sel2[pair, 64 * pair:64 * pair + C, 0:64] = 1.0
        sel2[pair, 64 * pair + 32:64 * pair + 32 + C, 64:128] = 1.0
    sel2_d = nc.inline_tensor(
        np.ascontiguousarray(sel2.transpose(1, 0, 2)).astype(ml_dtypes.bfloat16),
        "sel2")
    ident_d = nc.inline_tensor(np.eye(128, dtype=ml_dtypes.bfloat16), "ident")
    onescol_d = nc.inline_tensor(np.ones((128, 1), ml_dtypes.bfloat16), "onescol")
    onesrow_d = nc.inline_tensor(np.ones((1, 128), ml_dtypes.bfloat16), "onesrow")
    kv_scratch = [nc.dram_tensor(f"kvs{j}", [128, 2048], BF16) for j in range(2)]

    with tile.TileContext(nc) as tc:
        _emit(nc, tc, locals())
    nc.compile()
    return nc


def _emit(nc, tc, d):
    from contextlib import ExitStack

    with ExitStack() as ctx:
        ep = ctx.enter_context

        # ---------- persistent pools ----------
        consts = ep(tc.tile_pool(name="consts", bufs=1))
        wpool = ep(tc.tile_pool(name="wpool", bufs=1))      # WoT
        qkv = ep(tc.tile_pool(name="qkv", bufs=1))          # qT/kT/vnat
        qkv2 = ep(tc.tile_pool(name="qkv2", bufs=1))        # knat, row-0 copies
        wstore = ep(tc.tile_pool(name="wstore", bufs=1))    # w16/wT/carry/sk/sv
        gstore = ep(tc.tile_pool(name="gstore", bufs=1))    # masked G, all chunks
        onum = ep(tc.tile_pool(name="onum", bufs=1))        # o_num, rb_all
        sb2 = ep(tc.tile_pool(name="sb2", bufs=3))          # per-chunk sbuf
        otp = ep(tc.tile_pool(name="otp", bufs=1))          # oT final
        outp = ep(tc.tile_pool(name="outp", bufs=3))        # out staging

        # ---------- consts ----------
        u4_t = consts.tile([128, 4 * T], BF16, tag="u4")
        nc.sync.dma_start(out=u4_t, in_=d["u4_d"].ap())
        u128 = u4_t[:, 0:T]
        sel2_t = consts.tile([128, 2, 128], BF16, tag="sel2")
        nc.sync.dma_start(out=sel2_t, in_=d["sel2_d"].ap())
        ident_t = consts.tile([128, 128], BF16, tag="ident")
        nc.sync.dma_start(out=ident_t, in_=d["ident_d"].ap())
        onescol_t = consts.tile([128, 1], BF16, tag="onescol")
        nc.sync.dma_start(out=onescol_t, in_=d["onescol_d"].ap())
        onesrow_t = consts.tile([1, 128], BF16, tag="onesrow")
        nc.sync.dma_start(out=onesrow_t, in_=d["onesrow_d"].ap())
        qcT_t = consts.tile([64, HPC, 32], BF16, tag="qcT")
        nc.sync.dma_start(out=qcT_t, in_=d["qcT_d"].ap())
        bv_t = consts.tile([1, HD], BF16, tag="bv")
        nc.sync.dma_start(out=bv_t, in_=d["bv_d"].ap())

        # biases as (128, 2) per-partition columns (q/k only; v uses a row)
        def bias_tile(name):
            t = consts.tile([128, 2], F32, tag=name)
            nc.sync.dma_start(
                out=t, in_=d[name + "_d"].ap().rearrange("(m p) -> p m", p=128))
            return t
        bq_t, bk_t = bias_tile("bq"), bias_tile("bk")

        # WoT resident (bf16): (128, 2, D)
        woT_t = wpool.tile([128, 2, D], BF16, tag="woT")
        wor = d["woT_d"].ap().rearrange("(kt p) j -> kt p j", p=128)
        for kt in range(2):
            nc.sync.dma_start(out=woT_t[:, kt, :], in_=wor[kt])

        # ---------- projections (all bf16) ----------
        qT = [qkv.tile([128, S], BF16, tag=f"qT{m}", name=f"qT{m}") for m in range(2)]
        kT = [qkv.tile([128, S], BF16, tag=f"kT{m}", name=f"kT{m}") for m in range(2)]
        vnat = qkv.tile([128, NCH, HD], BF16, tag="vnat", name="vnat")

        with tc.tile_pool(name="xw", bufs=1) as xw, \
                tc.tile_pool(name="ppj", bufs=4, space="PSUM") as ppj:
            # DMA order = consumption order: wk weights FIRST (the first
            # matmul wave needs wk[0]+xT[0], not the whole 4MB of x), then
            # x tiles, then wq/wv (consumed ~25us later).
            xT_t = xw.tile([128, KT, S], BF16, tag="xT")
            xr = d["xT_d"].ap().rearrange("(kt p) s -> kt p s", p=128)
            wts = {}
            for nm in ("wk", "wq", "wv"):
                wts[nm] = xw.tile([128, KT, HD], BF16, tag=nm, name=nm)
            wrs = {nm: d[nm + "T_d"].ap().rearrange("(kt p) j -> kt p j", p=128)
                   for nm in ("wk", "wq", "wv")}
            for kt in range(KT):
                nc.sync.dma_start(out=wts["wk"][:, kt, :], in_=wrs["wk"][kt])
            for kt in range(KT):
                nc.sync.dma_start(out=xT_t[:, kt, :], in_=xr[kt])
            for nm in ("wq", "wv"):
                for kt in range(KT):
                    nc.sync.dma_start(out=wts[nm][:, kt, :], in_=wrs[nm][kt])

            # row-0 copies of odd heads' q/k (PE row-position changes between
            # back-to-back matmuls are fatal on HW; keep every operand at
            # row 0). Interleaved per n-tile so the copies finish with the
            # projections and the PE never gaps (HAM throttle) afterwards.
            qTo = [qkv2.tile([64, S], BF16, tag=f"qTo{j}", name=f"qTo{j}")
                   for j in range(2)]
            kTo = [qkv2.tile([64, S], BF16, tag=f"kTo{j}", name=f"kTo{j}")
                   for j in range(2)]
            knat = [qkv2.tile([128, NCH, 128], BF16, tag=f"knat{m}",
                              name=f"knat{m}") for m in range(2)]
            kvs = d["kv_scratch"]

            # kt-OUTER waves of 4 psum tiles: the first matmuls need only
            # xT[0] + w[0] (~4us into the DMA stream), not the full load --
            # PE starts ~30us earlier and the dense stream warms the clock.
            epi = 0

            def qk_waves(nm, dst, dsto, bias):
                nonlocal epi
                for m in range(2):
                    pss = [ppj.tile([128, 512], F32, tag="pj", name=f"pj{n}")
                           for n in range(4)]
                    for kt in range(KT):
                        for n in range(4):
                            nc.tensor.matmul(
                                pss[n], wts[nm][:, kt, 128 * m:128 * m + 128],
                                xT_t[:, kt, 512 * n:512 * n + 512],
                                start=(kt == 0), stop=(kt == KT - 1))
                    for n in range(4):
                        sl = slice(512 * n, 512 * n + 512)
                        if epi % 2 == 0:
                            nc.vector.tensor_scalar(
                                out=dst[m][:, sl], in0=pss[n],
                                scalar1=bias[:, m:m + 1], scalar2=None,
                                op0=mybir.AluOpType.add)
                        else:
                            nc.scalar.activation(
                                out=dst[m][:, sl], in_=pss[n],
                                func=IDENT, bias=bias[:, m:m + 1])
                        nc.sync.dma_start(out=dsto[m][:, sl],
                                          in_=dst[m][64:128, sl])
                        epi += 1

            # k first, then q, then v
            qk_waves("wk", kT, kTo, bk_t)
            # kick the k natural-layout DRAM round-trip early so the
            # xbar transpose lands while v projects
            for m in range(2):
                nc.sync.dma_start(out=kvs[m].ap(), in_=kT[m][:, :])
                nc.sync.dma_start_transpose(out=knat[m], in_=kvs[m].ap())
            qk_waves("wq", qT, qTo, bq_t)

            # v directly in natural (s, hd) layout: out = xT.T @ wvT
            for wv in range(4):
                vps = [ppj.tile([128, HD], F32, tag="vp", name=f"vp{j}")
                       for j in range(4)]
                for kt in range(KT):
                    for j in range(4):
                        sc = 4 * wv + j
                        nc.tensor.matmul(
                            vps[j], xT_t[:, kt, T * sc:T * sc + T],
                            wts["wv"][:, kt, :],
                            start=(kt == 0), stop=False)
                for j in range(4):
                    sc = 4 * wv + j
                    nc.tensor.matmul(vps[j], onesrow_t, bv_t,
                                     start=False, stop=True)
                    if sc % 2 == 0:
                        nc.vector.tensor_copy(out=vnat[:, sc, :], in_=vps[j])
                    else:
                        nc.scalar.copy(out=vnat[:, sc, :], in_=vps[j])

        def qTh(h):
            return qT[h // 2][0:64, :] if h % 2 == 0 else qTo[h // 2][:, :]

        def kTh(h):
            return kT[h // 2][0:64, :] if h % 2 == 0 else kTo[h // 2][:, :]

        # ---------- G = (k^T q) o U for all chunks (keeps PE dense) ----------
        gmt_all = gstore.tile([128, NCH, 4 * T], BF16, tag="gmt")
        with tc.tile_pool(name="pg", bufs=2, space="PSUM") as pg:
            for i in range(NCH):
                ch = slice(T * i, T * i + T)
                gt = pg.tile([128, 512], F32, tag="gt")
                for h in range(HPC):
                    nc.tensor.matmul(
                        gt[:, 128 * h:128 * h + 128],
                        kTh(h)[:, ch], qTh(h)[:, ch], start=True, stop=True)
                nc.vector.tensor_mul(gmt_all[:, i, :], gt, u4_t)

        # ---------- w16, packed wT, den carries, SK/SV prefixes ----------
        w16 = wstore.tile([C, HPC, S], BF16, tag="w16")
        # wT packed (t, 32h+c): cols 16-31 of each 32-group hold exp(0)=1
        # from the zero-padded qc -- written, never read back.
        wT_pad = wstore.tile([128, NCH, HPC, 32], BF16, tag="wTp")
        carry_cols = wstore.tile([128, NCH], F32, tag="carryc")
        # sk padded 16->32 (pad cols = k-sums against the wT pad ones;
        # written, only feeds never-read pad rows downstream)
        sk_all = wstore.tile([64, NCH, HPC, 32], BF16, tag="sk")
        sv_all = wstore.tile([C, NCH, HPC, DH], BF16, tag="sv")

        w16p = gstore.tile([128, S], BF16, tag="w16p")
        with tc.tile_pool(name="ph3a", bufs=3, space="PSUM") as ph3a:
            # w16 = exp(qc . k): packed (32h+c) matmuls + ONE exp per
            # 512-slice (Act time ~ free size, so packing is 4x cheaper),
            # then 4 sbuf-to-sbuf DMAs remap rows to the (c, h, s) layout.
            for n in range(4):
                ps = ph3a.tile([128, 512], F32, tag="w16ps")
                for h in range(HPC):
                    nc.tensor.matmul(
                        ps[32 * h:32 * h + 32, :], qcT_t[:, h, :],
                        kTh(h)[:, 512 * n:512 * n + 512],
                        start=True, stop=True, tile_position=(0, 32 * h))
                nc.scalar.activation(
                    out=w16p[:, 512 * n:512 * n + 512], in_=ps, func=EXP)
            for h in range(HPC):
                nc.sync.dma_start(out=w16[:, h, :],
                                  in_=w16p[32 * h:32 * h + C, :])

        with tc.tile_pool(name="ph3b", bufs=3, space="PSUM") as ph3b:
            # wT (t, 32h+c) = exp(k . qc_pad), dense matmul pass
            for i in range(NCH):
                ch = slice(T * i, T * i + T)
                tp = ph3b.tile([128, HPC, 32], F32, tag="tp")
                for h in range(HPC):
                    nc.tensor.matmul(
                        tp[:, h, :], kTh(h)[:, ch], qcT_t[:, h, :],
                        start=True, stop=True)
                nc.scalar.activation(out=wT_pad[:, i], in_=tp, func=EXP)
            # cs -> den carry column chain
            cs_prev = None
            for i in range(NCH - 1):
                cs = ph3b.tile([128, 1], F32, tag="cs")
                nc.tensor.matmul(cs, wT_pad[:, i], onescol_t,
                                 start=True, stop=True)
                if cs_prev is None:
                    nc.scalar.copy(out=carry_cols[:, 1:2], in_=cs)
                else:
                    nc.vector.tensor_add(carry_cols[:, i + 1:i + 2], cs,
                                         carry_cols[:, i:i + 1])
                cs_prev = cs

        with tc.tile_pool(name="ph3c", bufs=2, space="PSUM") as ph3c:
            # SK/SV chunk deltas + exclusive prefixes (slot i = state
            # before chunk i; slot 0 unused)
            skd_prev = None
            for i in range(NCH - 1):
                skd = ph3c.tile([64, HPC, 32], F32, tag="skd")
                svd = ph3c.tile([C, HPC, DH], F32, tag="svd")
                for h in range(HPC):
                    nc.tensor.matmul(
                        skd[:, h, :],
                        knat[h // 2][:, i, 64 * (h % 2):64 * (h % 2) + 64],
                        wT_pad[:, i, h, :], start=True, stop=True)
                    nc.tensor.matmul(
                        svd[:, h, :], wT_pad[:, i, h, 0:C],
                        vnat[:, i, 64 * h:64 * h + 64], start=True, stop=True)
                if skd_prev is None:
                    nc.scalar.copy(out=sk_all[:, 1], in_=skd)
                    nc.scalar.copy(out=sv_all[:, 1], in_=svd)
                else:
                    nc.vector.tensor_add(sk_all[:, i + 1], skd, sk_all[:, i])
                    nc.vector.tensor_add(sv_all[:, i + 1], svd, sv_all[:, i])
                skd_prev = skd

        # an state term SK_i^T q_i for all chunks, packed (32h+c, t) --
        # computed here and injected into the loop's an psum group with a
        # single identity matmul (1 matmul/chunk instead of 4 in-loop).
        st_all = wstore.tile([128, NCH, T], BF16, tag="st")
        with tc.tile_pool(name="ph3d", bufs=3, space="PSUM") as ph3d:
            for i in range(1, NCH):
                ch = slice(T * i, T * i + T)
                stp = ph3d.tile([128, T], F32, tag="stp")
                for h in range(HPC):
                    nc.tensor.matmul(
                        stp[32 * h:32 * h + 32, :], sk_all[:, i, h, :],
                        qTh(h)[:, ch], start=True, stop=True,
                        tile_position=(0, 32 * h))
                nc.scalar.copy(out=st_all[:, i, :], in_=stp)

        # ---------- PE heater ----------
        # The HAM clock gate un-throttles PE (1.2 -> 2.4 GHz) only after
        # ~3.4us of CONTINUOUS matmul activity, and re-throttles after
        # ~3.4us of continuous idle. The side phases above leave 4-5us PE
        # gaps (throttled); the loop's own gaps are <2.2us. One ~3.8us
        # back-to-back burst here re-arms full clock for the whole loop.
        with tc.tile_pool(name="pheat", bufs=1, space="PSUM") as pheat:
            hps = pheat.tile([128, 512], F32, tag="heat")
            for _ in range(18):
                nc.tensor.matmul(hps, u4_t[:, 0:128], u4_t,
                                 start=True, stop=True)
            heat_sb = consts.tile([1, 1], F32, tag="heatout")
            nc.scalar.copy(out=heat_sb, in_=hps[0:1, 0:1])

        # ---------- main chunk loop, software-pipelined ----------
        # Per-engine sequencers run IN ORDER, so a stage whose inputs are
        # still in flight head-of-line-blocks everything behind it. Stage
        # the emission: C(i-2) out-proj, B(i-1) PW/mix, A(i) den/att --
        # each stage's inputs were produced >= 1 iteration earlier.
        o_nm = onum.tile([128, 2, S], BF16, tag="o_nm")
        rb_all = onum.tile([128, 2, S], BF16, tag="rb_all")

        pda = ep(tc.tile_pool(name="pda", bufs=3, space="PSUM"))
        pbig = ep(tc.tile_pool(name="pbig", bufs=3, space="PSUM"))  # pw+mr
        pop = ep(tc.tile_pool(name="pop", bufs=2, space="PSUM"))

        ctiles = {}

        def stage_a(i):
            ch = slice(T * i, T * i + T)
            # den (packed 32h+c) in ONE matmul; att numerator per head into
            # the same packed layout via tile_position column offsets.
            da = pda.tile([128, 2, T], F32, tag="da")
            den_b = da[:, 0, :]
            an_b = da[:, 1, :]
            nc.tensor.matmul(den_b, wT_pad[:, i], u128, start=True, stop=True)
            for h in range(HPC):
                nc.tensor.matmul(
                    an_b[32 * h:32 * h + 32, :], wT_pad[:, i, h, :],
                    gmt_all[:, i, 128 * h:128 * h + 128],
                    start=True, stop=True, tile_position=(0, 32 * h))

            # softmax pieces, packed layout: 1/x as exp(-ln x), den carry
            # folded into the ln bias (per-partition column).
            lden = sb2.tile([128, T], F32, tag="lden")
            if i == 0:
                nc.scalar.activation(out=lden, in_=den_b, func=LN)
            else:
                nc.scalar.activation(out=lden, in_=den_b, func=LN,
                                     bias=carry_cols[:, i:i + 1])
            rden = sb2.tile([128, T], F32, tag="rden")
            nc.scalar.activation(out=rden, in_=lden, func=EXP, scale=-1.0)
            att = sb2.tile([128, T], F32, tag="att")
            if i > 0:
                # precomputed SK^T q added on DVE (PE is the loop
                # bottleneck at cold clock; DVE has slack)
                ans = sb2.tile([128, T], F32, tag="ans")
                nc.vector.tensor_add(ans, an_b, st_all[:, i, :])
                nc.vector.tensor_mul(att, ans, rden)
            else:
                nc.vector.tensor_mul(att, an_b, rden)
            e_b = sb2.tile([128, T], BF16, tag="e")
            nc.scalar.activation(out=e_b, in_=att, func=EXP)
            pdd_b = sb2.tile([128, T], BF16, tag="pdd")
            nc.gpsimd.tensor_mul(pdd_b, e_b, rden)

            # re-layout packed (32h+c, t) -> (c, h, t) for matmul consumers
            pdd_u = sb2.tile([C, HPC, T], BF16, tag="pddu")
            for h in range(HPC):
                nc.sync.dma_start(out=pdd_u[:, h, :],
                                  in_=pdd_b[32 * h:32 * h + C, :])
            ctiles[i] = (e_b, pdd_u)

        def oproj_block(c0):
            """Normalize + out-project + store chunks [c0, c0+4). Emitted
            >= 4 chunks behind the producers, so this is a dense ~3.4us
            matmul burst -- long-ready inputs, no head-of-line blocking --
            that also re-arms the 2.4 GHz HAM clock mid-loop."""
            bs = slice(T * c0, T * c0 + 4 * T)
            oT_b = otp.tile([128, 2, 4 * T], BF16, tag=f"oTb{(c0 // 4) % 2}")
            # all-SBUF bf16 multiply -> GpSimd (otherwise idle; DVE is loaded)
            nc.gpsimd.tensor_mul(oT_b, o_nm[:, :, bs], rb_all[:, :, bs])
            for sc in range(4):
                ob = outp.tile([128, D], BF16, tag="ob")
                for n2 in range(2):
                    ps = pop.tile([128, 512], F32, tag="ops")
                    for kt in range(2):
                        nc.tensor.matmul(
                            ps, oT_b[:, kt, T * sc:T * sc + T],
                            woT_t[:, kt, 512 * n2:512 * n2 + 512],
                            start=(kt == 0), stop=(kt == 1))
                    if n2 == 0:
                        nc.vector.tensor_copy(
                            out=ob[:, 512 * n2:512 * n2 + 512], in_=ps)
                    else:
                        nc.scalar.copy(
                            out=ob[:, 512 * n2:512 * n2 + 512], in_=ps)
                nc.sync.dma_start(
                    out=d["out_d"].ap()[T * (c0 + sc):T * (c0 + sc) + T, :],
                    in_=ob)

        def stage_b(i):
            ch = slice(T * i, T * i + T)
            e_b, pdd_u = ctiles[i]
            # PW (t, s) + mask
            pw = pbig.tile([128, 512], F32, tag="big")
            for h in range(HPC):
                nc.tensor.matmul(
                    pw[:, 128 * h:128 * h + 128], w16[:, h, ch],
                    pdd_u[:, h, :], start=True, stop=True)
            pwm = sb2.tile([128, 512], BF16, tag="pwm")
            nc.vector.tensor_mul(pwm, pw, u4_t)

            # o numerator (cols 0-255) + sums broadcast (cols 256-511)
            mr = pbig.tile([128, 512], F32, tag="big")
            for h in (0, 2, 1, 3):          # bp-stable order
                bp = 64 * (h % 2)
                cb = 128 * (h // 2)
                nc.tensor.matmul(
                    mr[bp:bp + 64, cb:cb + 128],
                    vnat[:, i, 64 * h:64 * h + 64],
                    pwm[:, 128 * h:128 * h + 128],
                    start=True, stop=(i == 0), tile_position=(0, bp))
                if i > 0:
                    nc.tensor.matmul(
                        mr[bp:bp + 64, cb:cb + 128],
                        sv_all[:, i, h, :], pdd_u[:, h, :],
                        start=False, stop=True, tile_position=(0, bp))
            for pair in range(2):
                nc.tensor.matmul(
                    mr[:, 256 + 128 * pair:384 + 128 * pair],
                    sel2_t[:, pair, :], e_b, start=True, stop=True)
            nc.vector.tensor_copy(out=o_nm[:, :, ch], in_=mr[:, 0:256])
            lsb = sb2.tile([128, 256], F32, tag="lsb")
            nc.scalar.activation(out=lsb, in_=mr[:, 256:512], func=LN)
            nc.scalar.activation(out=rb_all[:, :, ch], in_=lsb,
                                 func=EXP, scale=-1.0)

        for i in range(NCH + 2):
            if 2 <= i:
                stage_b(i - 2)          # lag 2: A(i)'s chain has ~2
            if i < NCH:                 # iterations to land before B reads it
                stage_a(i)
            if i in (10, 14):
                oproj_block(i - 10)     # 5+ chunks behind the producers
            elif i == 17:
                oproj_block(8)
        oproj_block(12)


def kernel(**inputs):
    global LAST_EXEC_NS
    x = np.ascontiguousarray(inputs["x"], np.float32)
    q_c, beta = np.asarray(inputs["q_c"]), np.asarray(inputs["beta"])
    Wq, bq = np.asarray(inputs["Wq"]), np.asarray(inputs["bq"])
    Wk, bk = np.asarray(inputs["Wk"]), np.asarray(inputs["bk"])
    Wv, bv = np.asarray(inputs["Wv"]), np.asarray(inputs["bv"])
    Wo, bo = np.asarray(inputs["Wo"]), np.asarray(inputs["bo"])

    if "nc" not in _CACHE:
        _CACHE["nc"] = _build()
    nc = _CACHE["nc"]

    BF = ml_dtypes.bfloat16
    # per-head query temperature 0.125*exp(-beta) folded into Wq/bq
    qscale = (0.125 * np.exp(-beta.astype(np.float64))).astype(np.float32)
    qs_hd = np.repeat(qscale, DH)                      # (D,) per out-dim
    Wq_s = Wq * qs_hd[:, None]
    bq_s = bq * qs_hd

    in_maps = []
    for core in range(8):
        b, hh = core // 4, core % 4
        hd = slice(hh * HD, hh * HD + HD)
        qct = np.zeros((64, HPC, 32), BF)
        qc_r = q_c[:, hd].reshape(C, HPC, DH)          # (c, h, d)
        for h in range(HPC):
            qct[:, h, 0:C] = qc_r[:, h, :].T.astype(BF)
        in_maps.append({
            "xT": np.ascontiguousarray(x[:, b, :].T.astype(BF)),
            "wqT": np.ascontiguousarray(Wq_s[hd, :].T.astype(BF)),
            "wkT": np.ascontiguousarray(Wk[hd, :].T.astype(BF)),
            "wvT": np.ascontiguousarray(Wv[hd, :].T.astype(BF)),
            "woT": np.ascontiguousarray(Wo[:, hd].T.astype(BF)),
            "qcT": qct,
            "bq": np.ascontiguousarray(bq_s[hd]),
            "bk": np.ascontiguousarray(bk[hd]),
            "bv": np.ascontiguousarray(bv[hd].astype(BF))[None, :],
        })

    trace = os.environ.get("TRN_PROFILE", "0") == "1"
    res = run_bass_kernel_spmd(nc, in_maps, list(range(8)), trace=trace)
    LAST_EXEC_NS = res.exec_time_ns

    out = np.zeros((S, B, D), np.float32)
    for core in range(8):
        out[:, core // 4, :] += res.results[core]["out_p"].astype(np.float32)
    out += bo[None, None, :].astype(np.float32)
    return out
